# revision 37
# baseline (speedup 1.0000x reference)
"""Trainium2 Bass kernel for nn_Adapter_SelfParam_CrossNonParam.

Bottleneck adapter: down-proj(1024->256)+exact GELU, self-attention over
the first 200 prompt tokens (4 heads), parameter-free cross-attention
prompt->tokens, concat, up-proj(256->1024), gate.

Sharding: data-parallel over batch B=64 across 8 NeuronCores (8 items
each); all weights replicated. No collectives.

V4: - x transposed to feature-major on HOST -> plain contiguous DMA
      loads; bf16 output stores (host upcasts).
    - self-attention computed k-major (S^T on PE): softmax denominator
      via ones-column in the V stationary, normalization broadcast on
      gpsimd; no P^T transposes, no reduce_sum. V produced directly in
      natural layout from the qkv projection.
    - ACT/DVE op count minimized: gate folded into up_W host-side,
      2-bank [128,1024] PSUM accumulators for down/up (one activation
      per half-GEMM), tokN pair-copies with persistent ones columns,
      paired cross-softmax exps, single fused out_proj bias add.
    - PSUM tags separated: "dA" (down/up GEMM) vs "ch" (chain) so
      up-proj tiles fill PE gaps while the softmax chain waits.
"""
import sys

sys.path.insert(0, "/opt/trn_rl_repo")

import numpy as np
import ml_dtypes
from contextlib import ExitStack

import concourse.bass as bass
import concourse.tile as tile
from concourse import bacc, mybir
from concourse.bass_utils import run_bass_kernel_spmd

F32 = mybir.dt.float32
BF16 = mybir.dt.bfloat16
AF = mybir.ActivationFunctionType
ADD = mybir.AluOpType.add

B, NTOK, C = 64, 1224, 1024
E, P, T = 256, 200, 1024
NH, HD = 4, 64
NCORES, BL = 8, 8           # cores, batch per core
ATT_SCALE = 1.0 / np.sqrt(HD)   # folded into q weights host-side
CROSS_SCALE = float(E) ** -0.5  # folded into cross-softmax exp scale

# prompt chunks (rows of the 200-token prompt)
PCH = [(0, 128), (128, 72)]


def build_nc():
    nc = bacc.Bacc("TRN2", target_bir_lowering=False, debug=False,
                   num_devices=NCORES)

    x_d = nc.dram_tensor("xb", [BL, 8, 128, NTOK], BF16,
                         kind="ExternalInput").ap()
    wpk_d = nc.dram_tensor("wpk", [128, 6272], BF16, kind="ExternalInput").ap()
    dbias_d = nc.dram_tensor("dbias", [128, 2], F32, kind="ExternalInput").ap()
    qkvb_d = nc.dram_tensor("qkvb", [128, 6], F32, kind="ExternalInput").ap()
    opb_d = nc.dram_tensor("opb", [128, 2], F32, kind="ExternalInput").ap()
    vbr_d = nc.dram_tensor("vbrow", [1, 256], F32, kind="ExternalInput").ap()
    out_d = nc.dram_tensor("out", [BL, NTOK, C], BF16,
                           kind="ExternalOutput").ap()

    with tile.TileContext(nc) as tc, ExitStack() as ctx:
        wp = ctx.enter_context(tc.tile_pool(name="wts", bufs=1))
        sb1 = ctx.enter_context(tc.tile_pool(name="sb1", bufs=1))
        sbr = ctx.enter_context(tc.tile_pool(name="sbr", bufs=2))
        sbx = ctx.enter_context(tc.tile_pool(name="sbx", bufs=4))
        pout = ctx.enter_context(tc.tile_pool(name="pout", bufs=3))
        psB = ctx.enter_context(tc.tile_pool(name="psB", bufs=3, space="PSUM"))
        psC = ctx.enter_context(tc.tile_pool(name="psC", bufs=5, space="PSUM"))

        # ---- resident weights: packed bf16 load on the sync ring ----
        wpk = wp.tile([128, 6272], BF16, tag="wpk")
        for c0 in range(0, 2048, 512):      # dwT first, chunked across queues
            nc.sync.dma_start(wpk[:, c0:c0 + 512], wpk_d[:, c0:c0 + 512])
        dwT = wpk[:, 0:2048]
        ipWT = [wpk[:, 2048:2816], wpk[:, 2816:3584]]
        opWT = [wpk[:, 3584:3840], wpk[:, 3840:4096]]
        upWT = [wpk[:, 4096:5120], wpk[:, 5120:6144]]
        idB = wpk[:, 6144:6272]
        # small f32 consts ride the idle gpsimd ring in parallel
        dbias = wp.tile([128, 2], F32, tag="dbias")
        nc.gpsimd.dma_start(dbias[:], dbias_d[:])
        qkvb = wp.tile([128, 6], F32, tag="qkvb")
        nc.gpsimd.dma_start(qkvb[:], qkvb_d[:])
        opb = wp.tile([128, 2], F32, tag="opb")
        nc.gpsimd.dma_start(opb[:], opb_d[:])
        vbrow = wp.tile([1, 4, 64], F32, tag="vbrow")
        nc.gpsimd.dma_start(vbrow[:1], vbr_d[:1])
        vbB = wp.tile([128, 4, 64], F32, tag="vbB")
        nc.gpsimd.partition_broadcast(vbB[:], vbrow[:1])
        # out_proj bias replicated along free dim: [128, m, 200]
        opbB = wp.tile([128, 2, P], F32, tag="opbB")
        nc.vector.memset(opbB[:], 0.0)
        for m in range(2):
            nc.vector.tensor_scalar_add(opbB[:, m], opbB[:, m],
                                        opb[:, m:m + 1])
        # persistent tok-natural tiles (+ones col), double-banked over b
        tokN = [wp.tile([128, 8, 257], BF16, tag=f"tokN{par}",
                        name=f"tokN{par}") for par in range(2)]
        nc.vector.memset(tokN[0][:, :, 256:257], 1.0)
        nc.vector.memset(tokN[1][:, :, 256:257], 1.0)

        # prompt^T for all 8 batch items: [256 feat, 8*200]
        promT = [wp.tile([128, BL * P], BF16, tag=f"promT{m}",
                         name=f"promT{m}") for m in range(2)]

        def transpose(out_ap, in_ap):
            pw = in_ap.partition_size()
            bp = in_ap.base_partition()
            nc.tensor.transpose(out_ap, in_ap, idB[bp:bp + pw, bp:bp + pw])

        xtiles = {}

        def emit_loads_h(b, h, split=False):
            if h == 0:
                xtiles[b] = [sbx.tile([128, NTOK], BF16, tag=f"xT{ci}",
                                      name=f"xT{b}_{ci}") for ci in range(8)]
            ts = xtiles[b]
            for ci in range(4 * h, 4 * h + 4):
                eng = nc.sync if ci % 2 == 0 else nc.scalar
                if split:
                    eng.dma_start(ts[ci][:, 0:512], x_d[b, ci, :, 0:512])
                    eng.dma_start(ts[ci][:, 512:NTOK], x_d[b, ci, :, 512:NTOK])
                else:
                    eng.dma_start(ts[ci][:], x_d[b, ci])

        def emit_loads(b, split=False):
            emit_loads_h(b, 0, split)
            emit_loads_h(b, 1, split)

        tokT = {}   # (b, m) -> [128, 1024] tok^T e-chunk

        def emit_down_m(b, m):
            xT = xtiles[b]
            pA1 = psB.tile([128, 512], F32, tag="dA")
            pA2 = psB.tile([128, 512], F32, tag="dA")
            pB = psC.tile([128, 200], F32, tag="ch")
            for ci in range(8):
                w = dwT[:, ci * 256 + m * 128:ci * 256 + (m + 1) * 128]
                st, sp = (ci == 0), (ci == 7)
                nc.tensor.matmul(pA1[:], w, xT[ci][:, 0:512],
                                 start=st, stop=sp)
                nc.tensor.matmul(pA2[:], w, xT[ci][:, 512:1024],
                                 start=st, stop=sp)
                nc.tensor.matmul(pB[:, 0:200], w, xT[ci][:, 1024:NTOK],
                                 start=st, stop=sp)
            if m == 1:
                del xtiles[b]
            tokTm = sb1.tile([128, T], BF16, tag=f"tokT{b}_{m}",
                             name=f"tokT{b}_{m}")
            tokT[(b, m)] = tokTm
            db = dbias[:, m:m + 1]
            nc.scalar.activation(promT[m][:, b * P:(b + 1) * P],
                                 pA1[:, 0:P], AF.Gelu, bias=db)
            nc.scalar.activation(tokTm[:, 0:312], pA1[:, P:512],
                                 AF.Gelu, bias=db)
            nc.scalar.activation(tokTm[:, 312:824], pA2[:],
                                 AF.Gelu, bias=db)
            nc.scalar.activation(tokTm[:, 824:1024], pB[:, 0:200],
                                 AF.Gelu, bias=db)

        # ---- qkv (q,k feature-major; v natural) per pair (bb, bb+1) ----
        qkvTd = {}
        vN = {}

        def emit_qkv_pair(bb):
            qts = [sbr.tile([128, 2 * P], BF16, tag=f"qkvT{bb}_{mi}",
                            name=f"qkvT{bb}_{mi}", bufs=1) for mi in range(4)]
            for mi in range(4):
                pq = psC.tile([128, 400], F32, tag="ch")
                for ki in range(2):
                    nc.tensor.matmul(
                        pq[:, :400],
                        ipWT[ki][:, mi * 128:(mi + 1) * 128],
                        promT[ki][:, bb * P:(bb + 2) * P],
                        start=(ki == 0), stop=(ki == 1))
                nc.scalar.activation(qts[mi][:], pq[:, :400], AF.Identity,
                                     bias=qkvb[:, mi:mi + 1])
            qkvTd[bb] = qts
            for bv in (bb, bb + 1):
                for pc, (p0, pw) in enumerate(PCH):
                    pv = psC.tile([128, 4, 64], F32, tag="ch")
                    for ki in range(2):
                        nc.tensor.matmul(
                            pv[:pw],
                            promT[ki][:, bv * P + p0:bv * P + p0 + pw],
                            ipWT[ki][:, 512:768],
                            start=(ki == 0), stop=(ki == 1))
                    vt = sb1.tile([128, 4, 65], BF16, tag=f"vN{bv}_{pc}",
                                  name=f"vN{bv}_{pc}")
                    nc.vector.tensor_add(vt[:pw, :, 0:64], pv[:pw], vbB[:pw])
                    nc.vector.memset(vt[:pw, :, 64:65], 1.0)
                    vN[(bv, pc)] = vt

        # ================= PHASE A: down-proj + gelu (all b) ============
        emit_loads(0, split=True)
        emit_loads(1, split=True)
        emit_loads(2)
        for b in range(BL):
            emit_down_m(b, 0)
            if b + 3 < BL:
                emit_loads_h(b + 3, 0)
            emit_down_m(b, 1)
            if b + 3 < BL:
                emit_loads_h(b + 3, 1)
            if b == 0:
                for c0 in range(2048, 6272, 1056):
                    c1 = min(c0 + 1056, 6272)
                    nc.sync.dma_start(wpk[:, c0:c1], wpk_d[:, c0:c1])
            if b % 2 == 1:
                emit_qkv_pair(b - 1)

        # ============ PHASE B: attention + cross + up (per b) ===========
        for b in range(BL):
            boff = b * P
            tokNb = tokN[b % 2]

            def emit_tokN(tp):     # transposes token pair (2*tp, 2*tp+1)
                psm = psC.tile([128, 2, 256], BF16, tag="ch")
                for j in range(2):
                    tt = 2 * tp + j
                    for m in range(2):
                        transpose(psm[:, j, m * 128:(m + 1) * 128],
                                  tokT[(b, m)][:, tt * 128:(tt + 1) * 128])
                nc.vector.tensor_copy(
                    tokNb[:, 2 * tp:2 * tp + 2, 0:256], psm[:])

            qkvT = qkvTd[b - b % 2]
            boff2 = (b % 2) * P

            def emit_ST(mi, hh):
                kvt = qkvT[2 + mi]
                qvt = qkvT[mi]
                hoff = hh * 64
                pss = psC.tile([128, 400], F32, tag="ch")
                for kc, (k0, kw) in enumerate(PCH):
                    nc.tensor.matmul(
                        pss[:kw, kc * 200:kc * 200 + P],
                        kvt[hoff:hoff + 64, boff2 + k0:boff2 + k0 + kw],
                        qvt[hoff:hoff + 64, boff2:boff2 + P],
                        start=True, stop=True)
                pet = sbr.tile([128, 400], BF16, tag="PeT", bufs=3)
                nc.scalar.activation(pet[:], pss[:], AF.Exp, bias=0.0)
                return pet

            saIn = [sbr.tile([128, P], BF16, tag=f"saIn{mi}",
                             name=f"saIn{mi}") for mi in range(2)]
            psmA = {}

            def emit_attnV(mi, hh, pet):
                u = 2 * mi + hh
                if hh == 0:
                    psmA[mi] = psC.tile([128, P], BF16, tag="ch",
                                        name=f"psmA{mi}")
                poq = psC.tile([128, 2, 65], F32, tag="ch")
                for qc, (q0, qw) in enumerate(PCH):
                    for kc, (k0, kw) in enumerate(PCH):
                        nc.tensor.matmul(
                            poq[:qw, qc, :],
                            pet[:kw, kc * 200 + q0:kc * 200 + q0 + qw],
                            vN[(b, kc)][:kw, u, :],
                            start=(kc == 0), stop=(kc == 1))
                rq = sbr.tile([128, 2, 1], F32, tag="rq")
                nc.vector.reciprocal(rq[:], poq[:, :, 64:65])
                siq = sbr.tile([128, 2, 64], BF16, tag="siq")
                for qc, (q0, qw) in enumerate(PCH):
                    nc.vector.tensor_scalar_mul(siq[:qw, qc, :],
                                                poq[:qw, qc, 0:64],
                                                rq[:qw, qc, :])
                    transpose(psmA[mi][hh * 64:(hh + 1) * 64, q0:q0 + qw],
                              siq[:qw, qc, :])
                if hh == 1:
                    nc.vector.tensor_copy(saIn[mi][:], psmA[mi][:])

            def emit_outproj():
                pm2 = psC.tile([128, 2, P], F32, tag="ch")
                for m in range(2):
                    for ki in range(2):
                        nc.tensor.matmul(pm2[:, m, :],
                                         opWT[ki][:, m * 128:(m + 1) * 128],
                                         saIn[ki][:],
                                         start=(ki == 0), stop=(ki == 1))
                st = sbr.tile([128, 2, P], BF16, tag="saT")
                nc.vector.tensor_add(st[:], pm2[:], opbB[:])
                return st

            def emit_logits(saT, j):    # logits+exp for token pair
                plt = psC.tile([128, 2, P], F32, tag="ch")
                for i in range(2):
                    tc_ = 2 * j + i
                    for ki in range(2):
                        nc.tensor.matmul(
                            plt[:, i, :],
                            tokT[(b, ki)][:, tc_ * 128:(tc_ + 1) * 128],
                            saT[:, ki, :],
                            start=(ki == 0), stop=(ki == 1))
                pct = sbr.tile([128, 2, P], BF16, tag=f"PcT{j}",
                               name=f"PcT{j}")
                nc.scalar.activation(pct[:], plt[:], AF.Exp,
                                     bias=0.0, scale=CROSS_SCALE)
                return pct

            poN = []

            def emit_crossout(PcT, pc):
                p0, pw = PCH[pc]
                pco = psC.tile([128, 257], F32, tag="ch")
                for tc_ in range(8):
                    nc.tensor.matmul(pco[:pw, :257],
                                     PcT[tc_ // 2][:, tc_ % 2, p0:p0 + pw],
                                     tokNb[:, tc_, :],
                                     start=(tc_ == 0), stop=(tc_ == 7))
                rr = sbr.tile([128, 1], F32, tag="rr")
                nc.vector.reciprocal(rr[:pw], pco[:pw, 256:257])
                pn = sbr.tile([128, E], BF16, tag=f"poN{pc}", name=f"poN{pc}")
                nc.scalar.activation(pn[:pw], pco[:pw, :E], AF.Copy,
                                     bias=0.0, scale=rr[:pw])
                poN.append(pn)

            def emit_poT():
                psm = psC.tile([128, 2, 256], BF16, tag="ch")
                for mi in range(2):
                    for pc, (p0, pw) in enumerate(PCH):
                        transpose(psm[:, mi, p0:p0 + pw],
                                  poN[pc][:pw, mi * 128:(mi + 1) * 128])
                pt = sbr.tile([128, 2, P], BF16, tag="poT")
                nc.vector.tensor_copy(pt[:], psm[:, :, 0:200])
                return pt

            def emit_uptile(bloc, on_act, src, off, mw, orow):
                outT = pout.tile([128, C], BF16, tag="outT")
                pUs = [psB.tile([128, 512], F32, tag="dA", name=f"pU{i}")
                       for i in range(2)]
                for ki in range(2):
                    lh = (src[:, ki, off:off + mw] if src is not None
                          else tokT[(bloc, ki)][:, off:off + mw])
                    for ncc in range(2):
                        nc.tensor.matmul(
                            pUs[ncc][:mw], lh,
                            upWT[ki][:, ncc * 512:(ncc + 1) * 512],
                            start=(ki == 0), stop=(ki == 1))
                for ncc in range(2):
                    o = outT[:mw, ncc * 512:(ncc + 1) * 512]
                    if on_act == (ncc == 0):
                        nc.scalar.activation(o, pUs[ncc][:mw], AF.Copy,
                                             bias=0.0)
                    else:
                        nc.vector.tensor_copy(o, pUs[ncc][:mw])
                nc.scalar.dma_start(out_d[bloc, orow:orow + mw, :],
                                    outT[:mw, :])

            def up_tok(tt):
                emit_uptile(b, tt % 2 == 0, None, tt * 128, 128,
                            P + tt * 128)

            # ---- interleaved emission: chain + fillers ----
            pet00 = emit_ST(0, 0)
            pet01 = emit_ST(0, 1)
            pet10 = emit_ST(1, 0)
            emit_attnV(0, 0, pet00)
            pet11 = emit_ST(1, 1)
            emit_attnV(0, 1, pet01)
            up_tok(0)
            emit_attnV(1, 0, pet10)
            emit_attnV(1, 1, pet11)
            saT = emit_outproj()
            emit_tokN(0)
            emit_tokN(1)
            pcts = [emit_logits(saT, 0), emit_logits(saT, 1)]
            emit_tokN(2)
            up_tok(1)
            emit_tokN(3)
            pcts += [emit_logits(saT, 2), emit_logits(saT, 3)]
            up_tok(2)
            emit_crossout(pcts, 0)
            up_tok(3)
            emit_crossout(pcts, 1)
            up_tok(4)
            poT = emit_poT()
            up_tok(5)
            up_tok(6)
            up_tok(7)
            emit_uptile(b, True, poT, 0, 128, 0)
            emit_uptile(b, False, poT, 128, 72, 128)

    nc.compile()
    return nc


_NC = None


def _get_nc():
    global _NC
    if _NC is None:
        _NC = build_nc()
    return _NC


def _prep_consts(down_W, down_b, up_W, up_b, in_proj_W, in_proj_b,
                 out_proj_W, out_proj_b, gate):
    f = np.float32
    down_W = np.asarray(down_W, f)
    in_proj_W = np.asarray(in_proj_W, f).copy()
    in_proj_b = np.asarray(in_proj_b, f).copy()
    vbrow = in_proj_b[2 * E:3 * E].reshape(1, 256).copy()
    in_proj_W[:E] *= ATT_SCALE
    in_proj_b[:E] *= ATT_SCALE
    gate = np.float32(np.asarray(gate))
    dwT = np.ascontiguousarray(
        down_W.T.reshape(8, 128, E).transpose(1, 0, 2).reshape(128, 2048))
    ipwt = np.ascontiguousarray(in_proj_W.T.reshape(2, 128, 768))
    opwt = np.ascontiguousarray(
        np.asarray(out_proj_W, f).T.reshape(2, 128, 256))
    upwt = np.ascontiguousarray(
        (np.asarray(up_W, f) * gate).T.reshape(2, 128, 1024))
    bf = ml_dtypes.bfloat16
    wpk = np.concatenate(
        [dwT, ipwt[0], ipwt[1], opwt[0], opwt[1], upwt[0], upwt[1],
         np.eye(128, dtype=f)], axis=1).astype(bf)
    return {
        "wpk": np.ascontiguousarray(wpk),
        "dbias": np.ascontiguousarray(np.asarray(down_b, f).reshape(2, 128).T),
        "qkvb": np.ascontiguousarray(in_proj_b.reshape(6, 128).T),
        "opb": np.ascontiguousarray(np.asarray(out_proj_b, f).reshape(2, 128).T),
        "vbrow": np.ascontiguousarray(vbrow),
    }


def run_kernel(inputs, trace=False):
    """Build in_maps, run on 8 cores, return (full_output, BassKernelResults)."""
    x = np.asarray(inputs["x"], np.float32)
    # feature-major on host: [B, 8ci, 128, NTOK], each (b, ci) contiguous
    xb = np.ascontiguousarray(
        x.astype(ml_dtypes.bfloat16)
        .reshape(B, NTOK, 8, 128).transpose(0, 2, 3, 1))
    consts = _prep_consts(
        inputs["down_W"], inputs["down_b"], inputs["up_W"], inputs["up_b"],
        inputs["in_proj_W"], inputs["in_proj_b"], inputs["out_proj_W"],
        inputs["out_proj_b"], inputs["gate"])
    in_maps = [dict(xb=xb[c * BL:(c + 1) * BL], **consts)
               for c in range(NCORES)]
    nc = _get_nc()
    res = run_bass_kernel_spmd(nc, in_maps, core_ids=list(range(NCORES)),
                               trace=trace)
    out = np.concatenate(
        [res.results[i]["out"] for i in range(NCORES)], axis=0
    ).astype(np.float32)
    up_b = np.asarray(inputs["up_b"], np.float32)
    gate = np.float32(np.asarray(inputs["gate"]))
    if np.any(up_b):
        out = out + gate * up_b
    return out, res


def kernel(**inputs):
    out, _ = run_kernel(inputs, trace=False)
    return out


# revision 38
# speedup vs baseline: 1.0206x; 1.0206x over previous
"""Trainium2 Bass kernel for nn_Adapter_SelfParam_CrossNonParam.

Bottleneck adapter: down-proj(1024->256)+exact GELU, self-attention over
the first 200 prompt tokens (4 heads), parameter-free cross-attention
prompt->tokens, concat, up-proj(256->1024), gate.

Sharding: data-parallel over batch B=64 across 8 NeuronCores (8 items
each); all weights replicated. No collectives.

V4: - x transposed to feature-major on HOST -> plain contiguous DMA
      loads; bf16 output stores (host upcasts).
    - self-attention computed k-major (S^T on PE): softmax denominator
      via ones-column in the V stationary, normalization broadcast on
      gpsimd; no P^T transposes, no reduce_sum. V produced directly in
      natural layout from the qkv projection.
    - ACT/DVE op count minimized: gate folded into up_W host-side,
      2-bank [128,1024] PSUM accumulators for down/up (one activation
      per half-GEMM), tokN pair-copies with persistent ones columns,
      paired cross-softmax exps, single fused out_proj bias add.
    - PSUM tags separated: "dA" (down/up GEMM) vs "ch" (chain) so
      up-proj tiles fill PE gaps while the softmax chain waits.
"""
import sys

sys.path.insert(0, "/opt/trn_rl_repo")

import numpy as np
import ml_dtypes
from contextlib import ExitStack

import concourse.bass as bass
import concourse.tile as tile
from concourse import bacc, mybir
from concourse.bass_utils import run_bass_kernel_spmd

F32 = mybir.dt.float32
BF16 = mybir.dt.bfloat16
AF = mybir.ActivationFunctionType
ADD = mybir.AluOpType.add

B, NTOK, C = 64, 1224, 1024
E, P, T = 256, 200, 1024
NH, HD = 4, 64
NCORES, BL = 8, 8           # cores, batch per core
ATT_SCALE = 1.0 / np.sqrt(HD)   # folded into q weights host-side
CROSS_SCALE = float(E) ** -0.5  # folded into cross-softmax exp scale

# prompt chunks (rows of the 200-token prompt)
PCH = [(0, 128), (128, 72)]


def build_nc():
    nc = bacc.Bacc("TRN2", target_bir_lowering=False, debug=False,
                   num_devices=NCORES)

    x_d = nc.dram_tensor("xb", [BL, 8, 128, NTOK], BF16,
                         kind="ExternalInput").ap()
    wpk_d = nc.dram_tensor("wpk", [128, 6272], BF16, kind="ExternalInput").ap()
    dbias_d = nc.dram_tensor("dbias", [128, 2], F32, kind="ExternalInput").ap()
    qkvb_d = nc.dram_tensor("qkvb", [128, 6], F32, kind="ExternalInput").ap()
    opb_d = nc.dram_tensor("opb", [128, 2], F32, kind="ExternalInput").ap()
    vbr_d = nc.dram_tensor("vbrow", [1, 256], F32, kind="ExternalInput").ap()
    out_d = nc.dram_tensor("out", [BL, NTOK, C], BF16,
                           kind="ExternalOutput").ap()

    with tile.TileContext(nc) as tc, ExitStack() as ctx:
        wp = ctx.enter_context(tc.tile_pool(name="wts", bufs=1))
        sb1 = ctx.enter_context(tc.tile_pool(name="sb1", bufs=1))
        sbr = ctx.enter_context(tc.tile_pool(name="sbr", bufs=2))
        sbx = ctx.enter_context(tc.tile_pool(name="sbx", bufs=4))
        pout = ctx.enter_context(tc.tile_pool(name="pout", bufs=3))
        psB = ctx.enter_context(tc.tile_pool(name="psB", bufs=3, space="PSUM"))
        psC = ctx.enter_context(tc.tile_pool(name="psC", bufs=5, space="PSUM"))

        # ---- resident weights: packed bf16 load on the sync ring ----
        wpk = wp.tile([128, 6272], BF16, tag="wpk")
        for c0 in range(0, 2048, 512):      # dwT first, chunked across queues
            nc.sync.dma_start(wpk[:, c0:c0 + 512], wpk_d[:, c0:c0 + 512])
        dwT = wpk[:, 0:2048]
        ipWT = [wpk[:, 2048:2816], wpk[:, 2816:3584]]
        opWT = [wpk[:, 3584:3840], wpk[:, 3840:4096]]
        upWT = [wpk[:, 4096:5120], wpk[:, 5120:6144]]
        idB = wpk[:, 6144:6272]
        # small f32 consts ride the idle gpsimd ring in parallel
        dbias = wp.tile([128, 2], F32, tag="dbias")
        nc.gpsimd.dma_start(dbias[:], dbias_d[:])
        qkvb = wp.tile([128, 6], F32, tag="qkvb")
        nc.gpsimd.dma_start(qkvb[:], qkvb_d[:])
        opb = wp.tile([128, 2], F32, tag="opb")
        nc.gpsimd.dma_start(opb[:], opb_d[:])
        vbrow = wp.tile([1, 4, 64], F32, tag="vbrow")
        nc.gpsimd.dma_start(vbrow[:1], vbr_d[:1])
        vbB = wp.tile([128, 4, 64], F32, tag="vbB")
        nc.gpsimd.partition_broadcast(vbB[:], vbrow[:1])
        # out_proj bias replicated along free dim: [128, m, 200]
        opbB = wp.tile([128, 2, P], F32, tag="opbB")
        nc.vector.memset(opbB[:], 0.0)
        for m in range(2):
            nc.vector.tensor_scalar_add(opbB[:, m], opbB[:, m],
                                        opb[:, m:m + 1])
        # persistent tok-natural tiles (+ones col), double-banked over b
        tokN = [wp.tile([128, 8, 257], BF16, tag=f"tokN{par}",
                        name=f"tokN{par}") for par in range(2)]
        nc.vector.memset(tokN[0][:, :, 256:257], 1.0)
        nc.vector.memset(tokN[1][:, :, 256:257], 1.0)

        # prompt^T for all 8 batch items: [256 feat, 8*200]
        promT = [wp.tile([128, BL * P], BF16, tag=f"promT{m}",
                         name=f"promT{m}") for m in range(2)]

        def transpose(out_ap, in_ap):
            pw = in_ap.partition_size()
            bp = in_ap.base_partition()
            nc.tensor.transpose(out_ap, in_ap, idB[bp:bp + pw, bp:bp + pw])

        xtiles = {}

        def emit_loads_h(b, h, split=False):
            if h == 0:
                xtiles[b] = [sbx.tile([128, NTOK], BF16, tag=f"xT{ci}",
                                      name=f"xT{b}_{ci}") for ci in range(8)]
            ts = xtiles[b]
            for ci in range(4 * h, 4 * h + 4):
                eng = nc.sync if ci % 2 == 0 else nc.scalar
                if split:
                    eng.dma_start(ts[ci][:, 0:512], x_d[b, ci, :, 0:512])
                    eng.dma_start(ts[ci][:, 512:NTOK], x_d[b, ci, :, 512:NTOK])
                else:
                    eng.dma_start(ts[ci][:], x_d[b, ci])

        def emit_loads(b, split=False):
            emit_loads_h(b, 0, split)
            emit_loads_h(b, 1, split)

        tokT = {}   # (b, m) -> [128, 1024] tok^T e-chunk

        def emit_down_m(b, m):
            xT = xtiles[b]
            pA1 = psB.tile([128, 512], F32, tag="dA")
            pA2 = psB.tile([128, 512], F32, tag="dA")
            pB = psC.tile([128, 200], F32, tag="ch")
            for ci in range(8):
                w = dwT[:, ci * 256 + m * 128:ci * 256 + (m + 1) * 128]
                st, sp = (ci == 0), (ci == 7)
                nc.tensor.matmul(pA1[:], w, xT[ci][:, 0:512],
                                 start=st, stop=sp)
                nc.tensor.matmul(pA2[:], w, xT[ci][:, 512:1024],
                                 start=st, stop=sp)
                nc.tensor.matmul(pB[:, 0:200], w, xT[ci][:, 1024:NTOK],
                                 start=st, stop=sp)
            if m == 1:
                del xtiles[b]
            tokTm = sb1.tile([128, T], BF16, tag=f"tokT{b}_{m}",
                             name=f"tokT{b}_{m}")
            tokT[(b, m)] = tokTm
            db = dbias[:, m:m + 1]
            nc.scalar.activation(promT[m][:, b * P:(b + 1) * P],
                                 pA1[:, 0:P], AF.Gelu, bias=db)
            nc.scalar.activation(tokTm[:, 0:312], pA1[:, P:512],
                                 AF.Gelu, bias=db)
            nc.scalar.activation(tokTm[:, 312:824], pA2[:],
                                 AF.Gelu, bias=db)
            nc.scalar.activation(tokTm[:, 824:1024], pB[:, 0:200],
                                 AF.Gelu, bias=db)

        # ---- qkv (q,k feature-major; v natural) per pair (bb, bb+1) ----
        qkvTd = {}
        vN = {}

        def emit_qkv_pair(bb):
            qts = [sbr.tile([128, 2 * P], BF16, tag=f"qkvT{bb}_{mi}",
                            name=f"qkvT{bb}_{mi}", bufs=1) for mi in range(4)]
            for mi in range(4):
                pq = psC.tile([128, 400], F32, tag="ch")
                for ki in range(2):
                    nc.tensor.matmul(
                        pq[:, :400],
                        ipWT[ki][:, mi * 128:(mi + 1) * 128],
                        promT[ki][:, bb * P:(bb + 2) * P],
                        start=(ki == 0), stop=(ki == 1))
                nc.scalar.activation(qts[mi][:], pq[:, :400], AF.Identity,
                                     bias=qkvb[:, mi:mi + 1])
            qkvTd[bb] = qts
            for bv in (bb, bb + 1):
                for pc, (p0, pw) in enumerate(PCH):
                    pv = psC.tile([128, 4, 64], F32, tag="ch")
                    for ki in range(2):
                        nc.tensor.matmul(
                            pv[:pw],
                            promT[ki][:, bv * P + p0:bv * P + p0 + pw],
                            ipWT[ki][:, 512:768],
                            start=(ki == 0), stop=(ki == 1))
                    vt = sb1.tile([128, 4, 65], BF16, tag=f"vN{bv}_{pc}",
                                  name=f"vN{bv}_{pc}")
                    nc.vector.tensor_add(vt[:pw, :, 0:64], pv[:pw], vbB[:pw])
                    nc.vector.memset(vt[:pw, :, 64:65], 1.0)
                    vN[(bv, pc)] = vt

        # ================= PHASE A: down-proj + gelu (all b) ============
        emit_loads(0, split=True)
        emit_loads(1, split=True)
        emit_loads(2)
        for b in range(BL):
            emit_down_m(b, 0)
            if b + 3 < BL:
                emit_loads_h(b + 3, 0)
            emit_down_m(b, 1)
            if b + 3 < BL:
                emit_loads_h(b + 3, 1)
            if b == 0:
                for c0 in range(2048, 6272, 1056):
                    c1 = min(c0 + 1056, 6272)
                    nc.sync.dma_start(wpk[:, c0:c1], wpk_d[:, c0:c1])
            if b % 2 == 1:
                emit_qkv_pair(b - 1)

        # ============ PHASE B: attention + cross + up (per b) ===========
        for b in range(BL):
            boff = b * P
            tokNb = tokN[b % 2]

            def emit_tokN(tp):     # transposes token pair (2*tp, 2*tp+1)
                psm = psC.tile([128, 2, 256], BF16, tag="ch")
                for j in range(2):
                    tt = 2 * tp + j
                    for m in range(2):
                        transpose(psm[:, j, m * 128:(m + 1) * 128],
                                  tokT[(b, m)][:, tt * 128:(tt + 1) * 128])
                nc.vector.tensor_copy(
                    tokNb[:, 2 * tp:2 * tp + 2, 0:256], psm[:])

            qkvT = qkvTd[b - b % 2]
            boff2 = (b % 2) * P

            def emit_ST(mi, hh):
                kvt = qkvT[2 + mi]
                qvt = qkvT[mi]
                hoff = hh * 64
                pss = psC.tile([128, 400], F32, tag="ch")
                for kc, (k0, kw) in enumerate(PCH):
                    nc.tensor.matmul(
                        pss[:kw, kc * 200:kc * 200 + P],
                        kvt[hoff:hoff + 64, boff2 + k0:boff2 + k0 + kw],
                        qvt[hoff:hoff + 64, boff2:boff2 + P],
                        start=True, stop=True)
                pet = sbr.tile([128, 400], BF16, tag="PeT", bufs=3)
                nc.scalar.activation(pet[:], pss[:], AF.Exp, bias=0.0)
                return pet

            saIn = [sbr.tile([128, P], BF16, tag=f"saIn{mi}",
                             name=f"saIn{mi}") for mi in range(2)]
            psmA = {}

            def emit_attnV(mi, hh, pet):
                u = 2 * mi + hh
                if hh == 0:
                    psmA[mi] = psC.tile([128, P], BF16, tag="ch",
                                        name=f"psmA{mi}")
                poq = psC.tile([128, 2, 65], F32, tag="ch")
                for qc, (q0, qw) in enumerate(PCH):
                    for kc, (k0, kw) in enumerate(PCH):
                        nc.tensor.matmul(
                            poq[:qw, qc, :],
                            pet[:kw, kc * 200 + q0:kc * 200 + q0 + qw],
                            vN[(b, kc)][:kw, u, :],
                            start=(kc == 0), stop=(kc == 1))
                rq = sbr.tile([128, 2, 1], F32, tag="rq")
                nc.vector.reciprocal(rq[:], poq[:, :, 64:65])
                siq = sbr.tile([128, 2, 64], BF16, tag="siq")
                for qc, (q0, qw) in enumerate(PCH):
                    nc.vector.tensor_scalar_mul(siq[:qw, qc, :],
                                                poq[:qw, qc, 0:64],
                                                rq[:qw, qc, :])
                    transpose(psmA[mi][hh * 64:(hh + 1) * 64, q0:q0 + qw],
                              siq[:qw, qc, :])
                if hh == 1:
                    nc.vector.tensor_copy(saIn[mi][:], psmA[mi][:])

            def emit_outproj():
                pm2 = psC.tile([128, 2, P], F32, tag="ch")
                for m in range(2):
                    for ki in range(2):
                        nc.tensor.matmul(pm2[:, m, :],
                                         opWT[ki][:, m * 128:(m + 1) * 128],
                                         saIn[ki][:],
                                         start=(ki == 0), stop=(ki == 1))
                st = sbr.tile([128, 2, P], BF16, tag="saT")
                nc.vector.tensor_add(st[:], pm2[:], opbB[:])
                return st

            def emit_logits(saT, j):    # logits+exp for token pair
                plt = psC.tile([128, 2, P], F32, tag="ch")
                for i in range(2):
                    tc_ = 2 * j + i
                    for ki in range(2):
                        nc.tensor.matmul(
                            plt[:, i, :],
                            tokT[(b, ki)][:, tc_ * 128:(tc_ + 1) * 128],
                            saT[:, ki, :],
                            start=(ki == 0), stop=(ki == 1))
                pct = sbr.tile([128, 2, P], BF16, tag=f"PcT{j}",
                               name=f"PcT{j}")
                nc.scalar.activation(pct[:], plt[:], AF.Exp,
                                     bias=0.0, scale=CROSS_SCALE)
                return pct

            poN = []

            def emit_crossout(PcT, pc):
                p0, pw = PCH[pc]
                pco = psC.tile([128, 257], F32, tag="ch")
                for tc_ in range(8):
                    nc.tensor.matmul(pco[:pw, :257],
                                     PcT[tc_ // 2][:, tc_ % 2, p0:p0 + pw],
                                     tokNb[:, tc_, :],
                                     start=(tc_ == 0), stop=(tc_ == 7))
                rr = sbr.tile([128, 1], F32, tag="rr")
                nc.vector.reciprocal(rr[:pw], pco[:pw, 256:257])
                pn = sbr.tile([128, E], BF16, tag=f"poN{pc}", name=f"poN{pc}")
                nc.scalar.activation(pn[:pw], pco[:pw, :E], AF.Copy,
                                     bias=0.0, scale=rr[:pw])
                poN.append(pn)

            def emit_poT():
                psm = psC.tile([128, 2, 256], BF16, tag="ch")
                for mi in range(2):
                    for pc, (p0, pw) in enumerate(PCH):
                        transpose(psm[:, mi, p0:p0 + pw],
                                  poN[pc][:pw, mi * 128:(mi + 1) * 128])
                pt = sbr.tile([128, 2, P], BF16, tag="poT")
                nc.vector.tensor_copy(pt[:], psm[:, :, 0:200])
                return pt

            def emit_uptile(bloc, on_act, src, off, mw, orow):
                outT = pout.tile([128, C], BF16, tag="outT")
                pUs = [psB.tile([128, 512], F32, tag="dA", name=f"pU{i}")
                       for i in range(2)]
                for ki in range(2):
                    lh = (src[:, ki, off:off + mw] if src is not None
                          else tokT[(bloc, ki)][:, off:off + mw])
                    for ncc in range(2):
                        nc.tensor.matmul(
                            pUs[ncc][:mw], lh,
                            upWT[ki][:, ncc * 512:(ncc + 1) * 512],
                            start=(ki == 0), stop=(ki == 1))
                for ncc in range(2):
                    o = outT[:mw, ncc * 512:(ncc + 1) * 512]
                    if on_act == (ncc == 0):
                        nc.scalar.activation(o, pUs[ncc][:mw], AF.Copy,
                                             bias=0.0)
                    else:
                        nc.vector.tensor_copy(o, pUs[ncc][:mw])
                nc.scalar.dma_start(out_d[bloc, orow:orow + mw, :],
                                    outT[:mw, :])

            def up_tok(tt):
                emit_uptile(b, tt % 2 == 0, None, tt * 128, 128,
                            P + tt * 128)

            # ---- interleaved emission: chain + fillers ----
            pet00 = emit_ST(0, 0)
            pet01 = emit_ST(0, 1)
            emit_attnV(0, 0, pet00)
            pet10 = emit_ST(1, 0)
            emit_attnV(0, 1, pet01)
            pet11 = emit_ST(1, 1)
            up_tok(0)
            emit_attnV(1, 0, pet10)
            emit_attnV(1, 1, pet11)
            saT = emit_outproj()
            emit_tokN(0)
            emit_tokN(1)
            pcts = [emit_logits(saT, 0), emit_logits(saT, 1)]
            emit_tokN(2)
            up_tok(1)
            emit_tokN(3)
            pcts += [emit_logits(saT, 2), emit_logits(saT, 3)]
            up_tok(2)
            emit_crossout(pcts, 0)
            up_tok(3)
            emit_crossout(pcts, 1)
            up_tok(4)
            poT = emit_poT()
            up_tok(5)
            up_tok(6)
            up_tok(7)
            emit_uptile(b, True, poT, 0, 128, 0)
            emit_uptile(b, False, poT, 128, 72, 128)

    nc.compile()
    return nc


_NC = None


def _get_nc():
    global _NC
    if _NC is None:
        _NC = build_nc()
    return _NC


def _prep_consts(down_W, down_b, up_W, up_b, in_proj_W, in_proj_b,
                 out_proj_W, out_proj_b, gate):
    f = np.float32
    down_W = np.asarray(down_W, f)
    in_proj_W = np.asarray(in_proj_W, f).copy()
    in_proj_b = np.asarray(in_proj_b, f).copy()
    vbrow = in_proj_b[2 * E:3 * E].reshape(1, 256).copy()
    in_proj_W[:E] *= ATT_SCALE
    in_proj_b[:E] *= ATT_SCALE
    gate = np.float32(np.asarray(gate))
    dwT = np.ascontiguousarray(
        down_W.T.reshape(8, 128, E).transpose(1, 0, 2).reshape(128, 2048))
    ipwt = np.ascontiguousarray(in_proj_W.T.reshape(2, 128, 768))
    opwt = np.ascontiguousarray(
        np.asarray(out_proj_W, f).T.reshape(2, 128, 256))
    upwt = np.ascontiguousarray(
        (np.asarray(up_W, f) * gate).T.reshape(2, 128, 1024))
    bf = ml_dtypes.bfloat16
    wpk = np.concatenate(
        [dwT, ipwt[0], ipwt[1], opwt[0], opwt[1], upwt[0], upwt[1],
         np.eye(128, dtype=f)], axis=1).astype(bf)
    return {
        "wpk": np.ascontiguousarray(wpk),
        "dbias": np.ascontiguousarray(np.asarray(down_b, f).reshape(2, 128).T),
        "qkvb": np.ascontiguousarray(in_proj_b.reshape(6, 128).T),
        "opb": np.ascontiguousarray(np.asarray(out_proj_b, f).reshape(2, 128).T),
        "vbrow": np.ascontiguousarray(vbrow),
    }


def run_kernel(inputs, trace=False):
    """Build in_maps, run on 8 cores, return (full_output, BassKernelResults)."""
    x = np.asarray(inputs["x"], np.float32)
    # feature-major on host: [B, 8ci, 128, NTOK], each (b, ci) contiguous
    xb = np.ascontiguousarray(
        x.astype(ml_dtypes.bfloat16)
        .reshape(B, NTOK, 8, 128).transpose(0, 2, 3, 1))
    consts = _prep_consts(
        inputs["down_W"], inputs["down_b"], inputs["up_W"], inputs["up_b"],
        inputs["in_proj_W"], inputs["in_proj_b"], inputs["out_proj_W"],
        inputs["out_proj_b"], inputs["gate"])
    in_maps = [dict(xb=xb[c * BL:(c + 1) * BL], **consts)
               for c in range(NCORES)]
    nc = _get_nc()
    res = run_bass_kernel_spmd(nc, in_maps, core_ids=list(range(NCORES)),
                               trace=trace)
    out = np.concatenate(
        [res.results[i]["out"] for i in range(NCORES)], axis=0
    ).astype(np.float32)
    up_b = np.asarray(inputs["up_b"], np.float32)
    gate = np.float32(np.asarray(inputs["gate"]))
    if np.any(up_b):
        out = out + gate * up_b
    return out, res


def kernel(**inputs):
    out, _ = run_kernel(inputs, trace=False)
    return out
